# revision 1
# baseline (speedup 1.0000x reference)
import numpy as np
import concourse.bass as bass
import concourse.tile as tile
from concourse import mybir
from concourse.bass_utils import run_bass_kernel_spmd
from concourse.masks import make_identity

P = 128
S = 2048
D = 512
U = 1024
NS = S // P      # 16 s-tiles
ND = D // P      # 4 d-blocks
NU = U // P      # 8 u-blocks
NEG = -60000.0
EPS = 1e-6


def _patched_drain_and_barrier(self, tick_clock, wait_clock):
    nc = self.nc
    probe = nc.sync.nop(nofuse=True, hint="drain_waits_probe")
    wait_clock.add_sem_waits(probe.ins, tile.ScopedClock({None: tick_clock.global_clock}))
    si = probe.ins.sync_info
    waits = list(si.on_wait) if si is not None else []
    assert self.sems is not None
    handles = {h.name: h for h in self.sems.allocated().values()}
    if len(waits) > 1:
        import bass_rust
        probe.ins.sync_info = bass_rust.SyncInfo(on_wait=waits[:1], on_update=[])
        for w in waits[1:]:
            h = handles.get(w.ant_name)
            assert h is not None, (w.ant_name, list(handles))
            nc.sync.wait_ge(h, w.wait_value)
    nc.sync.drain()
    nc.all_engine_barrier()
    popped = nc._tile_sem_poison_stack.pop()
    assert popped is self._sem_poison
    nc.clear_and_free_semaphores(list(self.sems.allocated().values()))
    nc.all_engine_barrier()


tile.TileContext._drain_and_barrier = _patched_drain_and_barrier

# The walrus backend in this toolchain rejects instructions carrying more
# than one semaphore wait ("Too many sync wait commands"). Split excess
# waits onto single-wait NoOp carriers on the same engine, which execute
# in order ahead of the real instruction.
_MAXW = 1
_orig_lower_ordered = tile.TileContext._lower_ordered_insts


def _patched_lower_ordered(self, ordered):
    nc = self.nc
    for insts in ordered.values():
        out = []
        for inst in insts:
            si = getattr(inst, "sync_info", None)
            eng = getattr(inst, "engine", None)
            if (si is not None and si.on_wait and len(si.on_wait) > _MAXW
                    and eng is not None
                    and not type(inst).__name__.startswith("BassTile")):
                waits = list(si.on_wait)
                for w in waits[:-_MAXW]:
                    out.append(mybir.InstNoOp(
                        name=nc.get_next_instruction_name(),
                        engine=eng,
                        ins=[],
                        outs=[],
                        bass_nofuse=True,
                        sync_info=mybir.SyncInfo(on_wait=[w], on_update=[]),
                    ))
                inst.sync_info = mybir.SyncInfo(
                    on_wait=waits[-_MAXW:], on_update=list(si.on_update))
            out.append(inst)
        insts[:] = out
    return _orig_lower_ordered(self, ordered)


tile.TileContext._lower_ordered_insts = _patched_lower_ordered

f32 = mybir.dt.float32
f16 = mybir.dt.float16


def _build():
    nc = bass.Bass()
    x_ext = nc.declare_dram_parameter("x", [S, D], f32, isOutput=False)
    bq_ext = nc.declare_dram_parameter("bq", [P, 2 * NU], f32, isOutput=False)
    wq_ext = nc.declare_dram_parameter("wq", [2 * D, U], f16, isOutput=False)
    wk_ext = nc.declare_dram_parameter("wk", [2 * D, U], f16, isOutput=False)
    wv_ext = nc.declare_dram_parameter("wv", [2 * D, U], f16, isOutput=False)
    wo_ext = nc.declare_dram_parameter("wo", [2 * U, D], f16, isOutput=False)
    out_ext = nc.declare_dram_parameter("out", [S, D], f32, isOutput=True)

    with tile.TileContext(nc) as tc:
        with tc.tile_pool(name="const", bufs=1) as cp, \
             tc.tile_pool(name="xnt", bufs=1) as xp, \
             tc.tile_pool(name="wp", bufs=1) as wp, \
             tc.tile_pool(name="wop", bufs=1) as wop, \
             tc.tile_pool(name="qkv", bufs=1) as qp, \
             tc.tile_pool(name="ln", bufs=2) as lp, \
             tc.tile_pool(name="xd", bufs=6) as xdp, \
             tc.tile_pool(name="att", bufs=2) as ap_, \
             tc.tile_pool(name="st", bufs=2) as sp, \
             tc.tile_pool(name="oacc", bufs=1) as op, \
             tc.tile_pool(name="outp", bufs=2) as up, \
             tc.tile_pool(name="mm", bufs=2, space="PSUM") as mmp, \
             tc.tile_pool(name="sc", bufs=2, space="PSUM") as scp, \
             tc.tile_pool(name="pv", bufs=1, space="PSUM") as pvp, \
             tc.tile_pool(name="tr", bufs=2, space="PSUM") as trp:

            ident = cp.tile([P, P], f16, tag="ident")
            make_identity(nc, ident[:])
            bqt = cp.tile([P, 2 * NU], f32, tag="bqt")
            nc.sync.dma_start(out=bqt[:], in_=bq_ext[:, :])
            eps = cp.tile([P, 1], f32, tag="eps")
            nc.vector.memset(eps[:], EPS)
            mask = cp.tile([P, 4 * D], f16, tag="mask")

            xnT = [xp.tile([P, S], f16, tag=f"xnt{j}", name=f"xnt{j}") for j in range(ND)]
            oacc = [op.tile([P, D], f16, tag=f"oacc{i}", name=f"oacc{i}") for i in range(NS)]

            dmaq = [nc.sync, nc.scalar, nc.gpsimd]
            xq = {0: 2, 3: 2}
            for _t in (1, 4, 6, 8, 10, 12, 14):
                xq[_t] = 0
            for _t in (2, 5, 7, 9, 11, 13, 15):
                xq[_t] = 1

            def emit_ln_tile(i):
                xt = xdp.tile([P, D], f32, tag="x", name="xt")
                dmaq[xq[i]].dma_start(out=xt[:], in_=x_ext[i * P:(i + 1) * P, :])
                stats = lp.tile([P, 6], f32, tag="bs", name="bs")
                nc.vector.bn_stats(out=stats[:], in_=xt[:])
                mv = lp.tile([P, 2], f32, tag="mv", name="mv")
                nc.vector.bn_aggr(out=mv[:], in_=stats[:])
                sd = lp.tile([P, 1], f32, tag="sd", name="sd")
                nc.scalar.activation(out=sd[:], in_=mv[:, 1:2],
                                     func=mybir.ActivationFunctionType.Sqrt,
                                     bias=eps[:], scale=1.0, alpha=0.0)
                nc.vector.reciprocal(out=sd[:], in_=sd[:])
                xh = lp.tile([P, D], f16, tag="xh", name="xh")
                nc.vector.tensor_scalar(out=xh[:], in0=xt[:],
                                        scalar1=mv[:, 0:1], scalar2=sd[:],
                                        op0=mybir.AluOpType.subtract,
                                        op1=mybir.AluOpType.mult)
                for j in range(ND):
                    tp = trp.tile([P, P], f16, tag="tr", name="tp")
                    nc.tensor.transpose(tp[:], xh[:, j * P:(j + 1) * P], ident[:])
                    nc.any.tensor_copy(out=xnT[j][:, i * P:(i + 1) * P], in_=tp[:])

            def load_w(w_ext_, h, engines=None):
                engines = engines or [nc.sync, nc.gpsimd, nc.sync, nc.gpsimd]
                wt = [wp.tile([P, U], f16, tag=f"w{j}", name=f"w{j}") for j in range(ND)]
                for j in range(ND):
                    engines[j].dma_start(
                        out=wt[j][:],
                        in_=w_ext_[h * D + j * P: h * D + (j + 1) * P, :])
                return wt

            def emit_proj_sl(wt, dst, sl, bcol=None):
                for u in range(NU):
                    mm = mmp.tile([P, 512], f32, tag="mm", name="mm")
                    for j in range(ND):
                        nc.tensor.matmul(mm[:],
                                         wt[j][:, u * P:(u + 1) * P],
                                         xnT[j][:, sl * 512:(sl + 1) * 512],
                                         start=(j == 0), stop=(j == ND - 1))
                    if bcol is None:
                        nc.any.tensor_copy(out=dst[u][:, sl * 512:(sl + 1) * 512], in_=mm[:])
                    else:
                        nc.any.tensor_scalar_add(out=dst[u][:, sl * 512:(sl + 1) * 512],
                                                 in0=mm[:],
                                                 scalar1=bqt[:, bcol + u:bcol + u + 1])

            def emit_v_prep(h):
                V = [qp.tile([P, U], f16, tag=f"v{t}", name=f"v{t}") for t in range(NS)]
                wt = load_w(wv_ext, h)
                return V, wt

            def emit_v_tile(V, wt, t):
                for us in range(2):
                    mm = mmp.tile([P, 512], f32, tag="mm", name="mm")
                    for j in range(ND):
                        nc.tensor.matmul(mm[:],
                                         xnT[j][:, t * P:(t + 1) * P],
                                         wt[j][:, us * 512:(us + 1) * 512],
                                         start=(j == 0), stop=(j == ND - 1))
                    nc.any.tensor_copy(out=V[t][:, us * 512:(us + 1) * 512], in_=mm[:])

            def load_wo(h):
                wo_t = [wop.tile([P, D], f16, tag=f"wo{ub}", name=f"wo{ub}") for ub in range(NU)]
                for ub in range(NU):
                    nc.gpsimd.dma_start(
                        out=wo_t[ub][:],
                        in_=wo_ext[h * U + ub * P: h * U + (ub + 1) * P, :])
                return wo_t

            def emit_scores(i, QT, KT):
                nch = i // 4 + 1
                Pt = ap_.tile([P, S], f16, tag="P", name="Pt")
                mneg = sp.tile([P, 4], f32, tag="mneg", name="mneg")
                rsum = sp.tile([P, 4], f32, tag="rsum", name="rsum")
                for c in range(nch):
                    w = (i % 4 + 1) * P if c == i // 4 else 512
                    sc = scp.tile([P, 512], f32, tag="sc", name="sc")
                    for u in range(NU):
                        nc.tensor.matmul(sc[:, 0:w],
                                         QT[u][:, i * P:(i + 1) * P],
                                         KT[u][:, c * 512:c * 512 + w],
                                         start=(u == 0), stop=(u == NU - 1))
                    if c == i // 4:
                        m = i % 4
                        nc.vector.tensor_add(out=sc[:, 0:w], in0=sc[:, 0:w],
                                             in1=mask[:, m * 512:m * 512 + w])
                    nc.vector.reduce_max(out=mneg[:, c:c + 1], in_=sc[:, 0:w],
                                         axis=mybir.AxisListType.X, negate=True)
                    nc.scalar.activation(out=Pt[:, c * 512:c * 512 + w], in_=sc[:, 0:w],
                                         func=mybir.ActivationFunctionType.Exp,
                                         bias=mneg[:, c:c + 1], scale=1.0,
                                         accum_out=rsum[:, c:c + 1])
                return Pt, mneg, rsum

            def emit_tail(h, i, Pt, mneg, rsum, V, wo_t, final=False):
                nch = i // 4 + 1
                # global softmax rescale: beta_c = exp(m_c - m_g) / Z
                mpos = sp.tile([P, 4], f32, tag="mpos", name="mpos")
                nc.vector.tensor_scalar_mul(out=mpos[:, 0:nch], in0=mneg[:, 0:nch],
                                            scalar1=-1.0)
                mgn = sp.tile([P, 1], f32, tag="mgn", name="mgn")
                nc.vector.reduce_max(out=mgn[:], in_=mpos[:, 0:nch],
                                     axis=mybir.AxisListType.X, negate=True)
                alph = sp.tile([P, 4], f32, tag="alph", name="alph")
                nc.scalar.activation(out=alph[:, 0:nch], in_=mneg[:, 0:nch],
                                     func=mybir.ActivationFunctionType.Exp,
                                     bias=mgn[:], scale=-1.0)
                pr = sp.tile([P, 4], f32, tag="pr", name="pr")
                nc.vector.tensor_mul(out=pr[:, 0:nch], in0=rsum[:, 0:nch],
                                     in1=alph[:, 0:nch])
                tot = sp.tile([P, 1], f32, tag="tot", name="tot")
                nc.vector.reduce_sum(out=tot[:], in_=pr[:, 0:nch],
                                     axis=mybir.AxisListType.X)
                nc.vector.reciprocal(out=tot[:], in_=tot[:])
                bt = sp.tile([P, 4], f32, tag="bt", name="bt")
                nc.vector.tensor_scalar_mul(out=bt[:, 0:nch], in0=alph[:, 0:nch],
                                            scalar1=tot[:])
                for c in range(nch):
                    w = (i % 4 + 1) * P if c == i // 4 else 512
                    nc.vector.tensor_scalar_mul(out=Pt[:, c * 512:c * 512 + w],
                                                in0=Pt[:, c * 512:c * 512 + w],
                                                scalar1=bt[:, c:c + 1])
                # transpose probs blocks 0..i
                pt = ap_.tile([P, S], f16, tag="pt", name="pt")
                for tb in range(i + 1):
                    tp = trp.tile([P, P], f16, tag="tr", name="tp")
                    nc.tensor.transpose(tp[:], Pt[:, tb * P:(tb + 1) * P], ident[:])
                    nc.any.tensor_copy(out=pt[:, tb * P:(tb + 1) * P], in_=tp[:])
                # probs @ V
                ht = ap_.tile([P, U], f16, tag="ht", name="ht")
                for us in range(2):
                    pv = pvp.tile([P, 512], f32, tag=f"pv{us}", name="pv")
                    for tb in range(i + 1):
                        nc.tensor.matmul(pv[:],
                                         pt[:, tb * P:(tb + 1) * P],
                                         V[tb][:, us * 512:(us + 1) * 512],
                                         start=(tb == 0), stop=(tb == i))
                    nc.any.tensor_copy(out=ht[:, us * 512:(us + 1) * 512], in_=pv[:])
                # transpose head-out blocks
                htt = ap_.tile([P, U], f16, tag="htt", name="htt")
                for ub in range(NU):
                    tp = trp.tile([P, P], f16, tag="tr", name="tp")
                    nc.tensor.transpose(tp[:], ht[:, ub * P:(ub + 1) * P], ident[:])
                    nc.any.tensor_copy(out=htt[:, ub * P:(ub + 1) * P], in_=tp[:])
                # output projection
                om = mmp.tile([P, 512], f32, tag="mm", name="om")
                for ub in range(NU):
                    nc.tensor.matmul(om[:],
                                     htt[:, ub * P:(ub + 1) * P],
                                     wo_t[ub][:],
                                     start=(ub == 0), stop=(ub == NU - 1))
                if h == 0:
                    nc.any.tensor_copy(out=oacc[i][:], in_=om[:])
                else:
                    of = up.tile([P, D], f32, tag="of", name="of")
                    nc.vector.tensor_add(out=of[:], in0=om[:], in1=oacc[i][:])
                    if final:
                        nc.sync.dma_start(out=out_ext[i * P:i * P + 64, :],
                                          in_=of[0:64, :])
                        nc.scalar.dma_start(out=out_ext[i * P + 64:(i + 1) * P, :],
                                            in_=of[64:128, :])
                    else:
                        nc.sync.dma_start(out=out_ext[i * P:(i + 1) * P, :], in_=of[:])

            # ---- LayerNorm interleaved with head-0 Q projection ----
            QT0 = [qp.tile([P, S], f16, tag=f"qt{u}", name=f"qt{u}") for u in range(NU)]
            for i in range(4):
                emit_ln_tile(i)
            wt0 = load_w(wq_ext, 0, engines=[nc.sync, nc.scalar, nc.sync, nc.scalar])
            nc.gpsimd.memset(mask[:], 0.0)
            for m in range(4):
                # keep 0 where j <= m*128 + r, else NEG
                nc.gpsimd.affine_select(
                    out=mask[:, m * 512:(m + 1) * 512],
                    in_=mask[:, m * 512:(m + 1) * 512],
                    compare_op=mybir.AluOpType.is_ge,
                    fill=NEG,
                    base=m * P,
                    pattern=[[-1, 512]],
                    channel_multiplier=1,
                )
            emit_proj_sl(wt0, QT0, 0, bcol=0)
            for g in range(1, 4):
                for i in range(4 * g, 4 * g + 4):
                    emit_ln_tile(i)
                emit_proj_sl(wt0, QT0, g, bcol=0)

            # ---- head 0: K, V, Wout ----
            wt = load_w(wk_ext, 0)
            KT0 = [qp.tile([P, S], f16, tag=f"kt{u}", name=f"kt{u}") for u in range(NU)]
            for sl in range(4):
                emit_proj_sl(wt, KT0, sl)
            wo_t0 = load_wo(0)
            V0, wtv = emit_v_prep(0)
            emit_v_tile(V0, wtv, 0)
            emit_v_tile(V0, wtv, 1)

            # ---- head 0 attention, software-pipelined by one stage;
            #      remaining V tiles interleaved as PE filler ----
            pend = None
            vnext = 2
            for i in range(NS):
                cur = (0, i) + emit_scores(i, QT0, KT0) + (V0, wo_t0)
                for _ in range(2):
                    if vnext < NS:
                        emit_v_tile(V0, wtv, vnext)
                        vnext += 1
                if pend is not None:
                    emit_tail(*pend)
                pend = cur

            # ---- head 1 Q/K projections fill the last softmax stall ----
            wt = load_w(wq_ext, 1)
            QT1 = [qp.tile([P, S], f16, tag=f"qt{u}", name=f"qt{u}") for u in range(NU)]
            for sl in range(4):
                emit_proj_sl(wt, QT1, sl, bcol=NU)
            wt = load_w(wk_ext, 1)
            KT1 = [qp.tile([P, S], f16, tag=f"kt{u}", name=f"kt{u}") for u in range(NU)]
            for sl in range(4):
                emit_proj_sl(wt, KT1, sl)
            emit_tail(*pend)  # head-0 i=15: must precede V1 overwrite of v tags
            wo_t1 = load_wo(1)
            V1, wtv = emit_v_prep(1)
            emit_v_tile(V1, wtv, 0)
            emit_v_tile(V1, wtv, 1)

            # ---- head 1 attention ----
            pend = None
            vnext = 2
            for i in range(NS):
                cur = (1, i) + emit_scores(i, QT1, KT1) + (V1, wo_t1)
                for _ in range(2):
                    if vnext < NS:
                        emit_v_tile(V1, wtv, vnext)
                        vnext += 1
                if pend is not None:
                    emit_tail(*pend)
                pend = cur
            emit_tail(*pend, final=True)
    return nc


_NC = None


def _get_nc():
    global _NC
    if _NC is None:
        _NC = _build()
    return _NC


def _run(inputs, trace=False):
    x = np.asarray(inputs["x"], dtype=np.float32)          # [4, 2048, 512]
    gamma = np.asarray(inputs["gamma"], dtype=np.float32).reshape(D)
    beta = np.asarray(inputs["beta"], dtype=np.float32).reshape(D)
    Wq = np.asarray(inputs["Wq"], dtype=np.float32)        # [4, 512, 1024]
    Wk = np.asarray(inputs["Wk"], dtype=np.float32)
    Wv = np.asarray(inputs["Wv"], dtype=np.float32)
    Wout = np.asarray(inputs["Wout"], dtype=np.float32)    # [4096, 512]

    # fold LN gamma into projection weights; beta terms:
    #  - K bias shifts each score row by a constant -> cancels in softmax
    #  - V bias passes through softmax (rows sum to 1) -> host-side constant
    #  - Q bias added in-kernel during psum evacuation
    Wqf = Wq * gamma[None, :, None]
    Wkf = Wk * gamma[None, :, None]
    Wvf = Wv * gamma[None, :, None]
    bq_all = np.einsum("d,hdu->hu", beta, Wq)              # [4, 1024]
    bv_all = np.einsum("d,hdu->hu", beta, Wv)              # [4, 1024]
    cvec = np.zeros(D, np.float32)
    for h in range(4):
        cvec += bv_all[h] @ Wout[h * U:(h + 1) * U]

    in_maps = []
    for c in range(8):
        b, hp = c // 2, c % 2
        bq = bq_all[2 * hp:2 * hp + 2].reshape(2, NU, P).transpose(2, 0, 1).reshape(P, 2 * NU)
        in_maps.append({
            "x": np.ascontiguousarray(x[b]),
            "bq": np.ascontiguousarray(bq),
            "wq": np.ascontiguousarray(Wqf[2 * hp:2 * hp + 2].reshape(2 * D, U)).astype(np.float16),
            "wk": np.ascontiguousarray(Wkf[2 * hp:2 * hp + 2].reshape(2 * D, U)).astype(np.float16),
            "wv": np.ascontiguousarray(Wvf[2 * hp:2 * hp + 2].reshape(2 * D, U)).astype(np.float16),
            "wo": np.ascontiguousarray(Wout[2 * hp * U:(2 * hp + 2) * U]).astype(np.float16),
        })
    res = run_bass_kernel_spmd(_get_nc(), in_maps, list(range(8)), trace=trace)
    out = np.empty((4, S, D), np.float32)
    for b in range(4):
        out[b] = res.results[2 * b]["out"] + res.results[2 * b + 1]["out"] + cvec[None, :]
    return out, res


def kernel(**inputs):
    out, _ = _run(inputs, trace=False)
    return out



# revision 7
# speedup vs baseline: 1.7937x; 1.7937x over previous
import numpy as np
import concourse.bass as bass
import concourse.tile as tile
from concourse import mybir
from concourse.bass_utils import run_bass_kernel_spmd
from concourse.masks import make_identity

P = 128
S = 2048
D = 512
U = 1024
NS = S // P      # 16 s-tiles
ND = D // P      # 4 d-blocks
NEG = -60000.0
EPS = 1e-6


def _patched_drain_and_barrier(self, tick_clock, wait_clock):
    nc = self.nc
    probe = nc.sync.nop(nofuse=True, hint="drain_waits_probe")
    wait_clock.add_sem_waits(probe.ins, tile.ScopedClock({None: tick_clock.global_clock}))
    si = probe.ins.sync_info
    waits = list(si.on_wait) if si is not None else []
    assert self.sems is not None
    handles = {h.name: h for h in self.sems.allocated().values()}
    if len(waits) > 1:
        import bass_rust
        probe.ins.sync_info = bass_rust.SyncInfo(on_wait=waits[:1], on_update=[])
        for w in waits[1:]:
            h = handles.get(w.ant_name)
            assert h is not None, (w.ant_name, list(handles))
            nc.sync.wait_ge(h, w.wait_value)
    nc.sync.drain()
    nc.all_engine_barrier()
    popped = nc._tile_sem_poison_stack.pop()
    assert popped is self._sem_poison
    nc.clear_and_free_semaphores(list(self.sems.allocated().values()))
    nc.all_engine_barrier()


tile.TileContext._drain_and_barrier = _patched_drain_and_barrier

# The walrus backend in this toolchain rejects instructions carrying more
# than one semaphore wait ("Too many sync wait commands"). Split excess
# waits onto single-wait NoOp carriers on the same engine, which execute
# in order ahead of the real instruction.
_MAXW = 1
_orig_lower_ordered = tile.TileContext._lower_ordered_insts


def _patched_lower_ordered(self, ordered):
    nc = self.nc
    for insts in ordered.values():
        out = []
        for inst in insts:
            si = getattr(inst, "sync_info", None)
            eng = getattr(inst, "engine", None)
            if (si is not None and si.on_wait and len(si.on_wait) > _MAXW
                    and eng is not None
                    and not type(inst).__name__.startswith("BassTile")):
                waits = list(si.on_wait)
                for w in waits[:-_MAXW]:
                    out.append(mybir.InstNoOp(
                        name=nc.get_next_instruction_name(),
                        engine=eng,
                        ins=[],
                        outs=[],
                        bass_nofuse=True,
                        sync_info=mybir.SyncInfo(on_wait=[w], on_update=[]),
                    ))
                inst.sync_info = mybir.SyncInfo(
                    on_wait=waits[-_MAXW:], on_update=list(si.on_update))
            out.append(inst)
        insts[:] = out
    return _orig_lower_ordered(self, ordered)


tile.TileContext._lower_ordered_insts = _patched_lower_ordered

f32 = mybir.dt.float32
f16 = mybir.dt.float16
bf16 = mybir.dt.bfloat16


def _build():
    nc = bass.Bass()
    # Per-core inputs (1 batch element, 2 heads):
    #   x   [S, D]    activations
    #   ub  [P, 2*ND] per-head score key-side bias (beta @ Wq @ (g*Wk)^T),
    #                 column h*ND+j holds entries d = j*128 + p
    #   a   [2D, D]   A_h = (g*Wq_h)(g*Wk_h)^T stacked over the 2 heads, f16
    #   n   [2D, D]   N_h = (g*Wv_h) Wout_h stacked, f16
    # scores = (z @ A + u) @ z^T ; out = sum_h probs_h @ (z @ N_h) / Z_h
    x_ext = nc.declare_dram_parameter("x", [S, D], f32, isOutput=False)
    ub_ext = nc.declare_dram_parameter("ub", [P, 2 * ND], f32, isOutput=False)
    a_ext = nc.declare_dram_parameter("a", [2 * D, D], f16, isOutput=False)
    n_ext = nc.declare_dram_parameter("n", [2 * D, D], f16, isOutput=False)
    out_ext = nc.declare_dram_parameter("out", [S, D], f32, isOutput=True)

    with tile.TileContext(nc) as tc:
        with tc.tile_pool(name="const", bufs=1) as cp, \
             tc.tile_pool(name="znt", bufs=1) as xp, \
             tc.tile_pool(name="wp", bufs=2) as wp, \
             tc.tile_pool(name="qkv", bufs=1) as qp, \
             tc.tile_pool(name="ln", bufs=2) as lp, \
             tc.tile_pool(name="xd", bufs=6) as xdp, \
             tc.tile_pool(name="att", bufs=2) as ap_, \
             tc.tile_pool(name="st", bufs=2) as sp, \
             tc.tile_pool(name="oacc", bufs=1) as op, \
             tc.tile_pool(name="outp", bufs=2) as up, \
             tc.tile_pool(name="mm", bufs=2, space="PSUM") as mmp, \
             tc.tile_pool(name="sc", bufs=2, space="PSUM") as scp, \
             tc.tile_pool(name="pv", bufs=1, space="PSUM") as pvp, \
             tc.tile_pool(name="tr", bufs=2, space="PSUM") as trp, \
             tc.tile_pool(name="trl", bufs=1, space="PSUM") as trlp:

            ident = cp.tile([P, P], f16, tag="ident")
            make_identity(nc, ident[:])
            identb = cp.tile([P, P], bf16, tag="identb")
            make_identity(nc, identb[:])
            ubt = cp.tile([P, 2 * ND], f32, tag="ubt")
            nc.sync.dma_start(out=ubt[:], in_=ub_ext[:, :])
            eps = cp.tile([P, 1], f32, tag="eps")
            nc.vector.memset(eps[:], EPS)
            mask = cp.tile([P, 4 * D], f16, tag="mask")

            zT = [xp.tile([P, S], f16, tag=f"zt{j}", name=f"zt{j}") for j in range(ND)]
            oacc = [op.tile([P, D], f32, tag=f"oacc{i}", name=f"oacc{i}") for i in range(NS)]
            qmT = [qp.tile([P, S], f16, tag=f"qmt{j}", name=f"qmt{j}") for j in range(ND)]
            vm = [qp.tile([P, D], bf16, tag=f"vm{t}", name=f"vm{t}") for t in range(NS)]

            dmaq = [nc.sync, nc.scalar, nc.gpsimd]
            xq = {0: 2, 3: 2}
            for _t in (1, 4, 6, 8, 10, 12, 14):
                xq[_t] = 0
            for _t in (2, 5, 7, 9, 11, 13, 15):
                xq[_t] = 1

            def emit_ln_tile(i):
                xt = xdp.tile([P, D], f32, tag="x", name="xt")
                dmaq[xq[i]].dma_start(out=xt[:], in_=x_ext[i * P:(i + 1) * P, :])
                stats = lp.tile([P, 6], f32, tag="bs", name="bs")
                nc.vector.bn_stats(out=stats[:], in_=xt[:])
                mv = lp.tile([P, 2], f32, tag="mv", name="mv")
                nc.vector.bn_aggr(out=mv[:], in_=stats[:])
                sd = lp.tile([P, 1], f32, tag="sd", name="sd")
                nc.scalar.activation(out=sd[:], in_=mv[:, 1:2],
                                     func=mybir.ActivationFunctionType.Sqrt,
                                     bias=eps[:], scale=1.0, alpha=0.0)
                nc.vector.reciprocal(out=sd[:], in_=sd[:])
                xh = lp.tile([P, D], f16, tag="xh", name="xh")
                nc.vector.tensor_scalar(out=xh[:], in0=xt[:],
                                        scalar1=mv[:, 0:1], scalar2=sd[:],
                                        op0=mybir.AluOpType.subtract,
                                        op1=mybir.AluOpType.mult)
                for j in range(ND):
                    tp = trlp.tile([P, P], f16, tag="tr", name="tp")
                    nc.tensor.transpose(tp[:], xh[:, j * P:(j + 1) * P], ident[:])
                    nc.any.tensor_copy(out=zT[j][:, i * P:(i + 1) * P], in_=tp[:])

            def load_w(w_ext_, h, tagc, engines=None):
                engines = engines or [nc.sync, nc.gpsimd, nc.sync, nc.gpsimd]
                wt = [wp.tile([P, D], f16, tag=f"{tagc}{k}", name=f"{tagc}{k}")
                      for k in range(ND)]
                for k in range(ND):
                    engines[k].dma_start(
                        out=wt[k][:],
                        in_=w_ext_[h * D + k * P: h * D + (k + 1) * P, :])
                return wt

            def emit_qm(h, at, g):
                # qmT[j][:, g*512:(g+1)*512] = A_h^T z^T + u  (d-tile j, s-chunk g)
                for j in range(ND):
                    mm = mmp.tile([P, D], f32, tag="mm", name="mm")
                    for k in range(ND):
                        nc.tensor.matmul(mm[:],
                                         at[k][:, j * P:(j + 1) * P],
                                         zT[k][:, g * D:(g + 1) * D],
                                         start=(k == 0), stop=(k == ND - 1))
                    nc.any.tensor_scalar_add(out=qmT[j][:, g * D:(g + 1) * D],
                                             in0=mm[:],
                                             scalar1=ubt[:, h * ND + j:h * ND + j + 1])

            def emit_vm(nt, t):
                # vm[t] = z N_h   (t-tile of rows)
                mm = mmp.tile([P, D], f32, tag="mm", name="mm")
                for k in range(ND):
                    nc.tensor.matmul(mm[:],
                                     zT[k][:, t * P:(t + 1) * P],
                                     nt[k][:, :],
                                     start=(k == 0), stop=(k == ND - 1))
                nc.any.tensor_copy(out=vm[t][:], in_=mm[:])

            def emit_scores(i):
                # scores row-tile i vs keys 0..(i+1)*128; exp without max-sub
                nch = i // 4 + 1
                Pt = ap_.tile([P, S], bf16, tag="P", name="Pt")
                rsum = sp.tile([P, 4], f32, tag="rsum", name="rsum")
                for c in range(nch):
                    w = (i % 4 + 1) * P if c == i // 4 else D
                    sc = scp.tile([P, D], f32, tag="sc", name="sc")
                    for k in range(ND):
                        nc.tensor.matmul(sc[:, 0:w],
                                         qmT[k][:, i * P:(i + 1) * P],
                                         zT[k][:, c * D:c * D + w],
                                         start=(k == 0), stop=(k == ND - 1))
                    if c == i // 4:
                        m = i % 4
                        nc.vector.tensor_add(out=sc[:, 0:w], in0=sc[:, 0:w],
                                             in1=mask[:, m * D:m * D + w])
                    nc.scalar.activation(out=Pt[:, c * D:c * D + w], in_=sc[:, 0:w],
                                         func=mybir.ActivationFunctionType.Exp,
                                         scale=1.0,
                                         accum_out=rsum[:, c:c + 1])
                return Pt, rsum

            def emit_tail(h, i, Pt, rsum, final=False):
                nch = i // 4 + 1
                # 1/Z
                tot = sp.tile([P, 1], f32, tag="tot", name="tot")
                if nch > 1:
                    nc.vector.reduce_sum(out=tot[:], in_=rsum[:, 0:nch],
                                         axis=mybir.AxisListType.X)
                    nc.vector.reciprocal(out=tot[:], in_=tot[:])
                else:
                    nc.vector.reciprocal(out=tot[:], in_=rsum[:, 0:1])
                # transpose probs blocks 0..i, 4 blocks per PSUM bank
                pt = ap_.tile([P, S], bf16, tag="pt", name="pt")
                for gr in range((i + 4) // 4):
                    tpw = min(4, i + 1 - gr * 4)
                    tp = trp.tile([P, D], bf16, tag="trb", name="tpb")
                    for q in range(tpw):
                        tb = gr * 4 + q
                        nc.tensor.matmul(tp[:, q * P:(q + 1) * P],
                                         Pt[:, tb * P:(tb + 1) * P], identb[:],
                                         is_transpose=True, skip_group_check=True)
                    nc.any.tensor_copy(out=pt[:, gr * D:gr * D + tpw * P],
                                       in_=tp[:, 0:tpw * P])
                # probs @ vm
                pv = pvp.tile([P, D], f32, tag="pv", name="pv")
                for tb in range(i + 1):
                    nc.tensor.matmul(pv[:],
                                     pt[:, tb * P:(tb + 1) * P],
                                     vm[tb][:],
                                     start=(tb == 0), stop=(tb == i))
                if h == 0:
                    nc.vector.tensor_scalar_mul(out=oacc[i][:], in0=pv[:],
                                                scalar1=tot[:])
                else:
                    of = up.tile([P, D], f32, tag="of", name="of")
                    nc.scalar.activation(out=of[:], in_=pv[:],
                                         func=mybir.ActivationFunctionType.Copy,
                                         scale=tot[:])
                    of2 = up.tile([P, D], f32, tag="of2", name="of2")
                    nc.vector.tensor_add(out=of2[:], in0=of[:], in1=oacc[i][:])
                    if final:
                        nc.sync.dma_start(out=out_ext[i * P:i * P + 64, :],
                                          in_=of2[0:64, :])
                        nc.scalar.dma_start(out=out_ext[i * P + 64:(i + 1) * P, :],
                                            in_=of2[64:128, :])
                    else:
                        nc.sync.dma_start(out=out_ext[i * P:(i + 1) * P, :], in_=of2[:])

            # ---- LayerNorm interleaved with head-0 qm ----
            at0 = load_w(a_ext, 0, "a", engines=[nc.sync, nc.scalar, nc.sync, nc.scalar])
            nc.gpsimd.memset(mask[:], 0.0)
            for m in range(4):
                # keep 0 where t <= m*128 + r, else NEG
                nc.gpsimd.affine_select(
                    out=mask[:, m * D:(m + 1) * D],
                    in_=mask[:, m * D:(m + 1) * D],
                    compare_op=mybir.AluOpType.is_ge,
                    fill=NEG,
                    base=m * P,
                    pattern=[[-1, D]],
                    channel_multiplier=1,
                )
            for g in range(4):
                for i in range(4 * g, 4 * g + 4):
                    emit_ln_tile(i)
                emit_qm(0, at0, g)

            # ---- head 0 attention, software-pipelined by one stage;
            #      vm tiles interleaved as PE filler ----
            nt0 = load_w(n_ext, 0, "n")
            emit_vm(nt0, 0)
            emit_vm(nt0, 1)
            pend = None
            vnext = 2
            for i in range(NS):
                cur = (0, i) + emit_scores(i)
                for _ in range(2):
                    if vnext < NS:
                        emit_vm(nt0, vnext)
                        vnext += 1
                if pend is not None:
                    emit_tail(*pend)
                pend = cur

            # ---- head 1 qm fills the last softmax stall ----
            at1 = load_w(a_ext, 1, "a")
            for g in range(4):
                emit_qm(1, at1, g)
            emit_tail(*pend)  # head-0 i=15: must precede vm overwrite
            nt1 = load_w(n_ext, 1, "n")
            emit_vm(nt1, 0)
            emit_vm(nt1, 1)

            # ---- head 1 attention ----
            pend = None
            vnext = 2
            for i in range(NS):
                cur = (1, i) + emit_scores(i)
                for _ in range(2):
                    if vnext < NS:
                        emit_vm(nt1, vnext)
                        vnext += 1
                if pend is not None:
                    emit_tail(*pend)
                pend = cur
            emit_tail(*pend, final=True)
    return nc


_NC = None


def _get_nc():
    global _NC
    if _NC is None:
        _NC = _build()
    return _NC


def _run(inputs, trace=False):
    x = np.asarray(inputs["x"], dtype=np.float32)          # [4, 2048, 512]
    gamma = np.asarray(inputs["gamma"], dtype=np.float32).reshape(D)
    beta = np.asarray(inputs["beta"], dtype=np.float32).reshape(D)
    Wq = np.asarray(inputs["Wq"], dtype=np.float32)        # [4, 512, 1024]
    Wk = np.asarray(inputs["Wk"], dtype=np.float32)
    Wv = np.asarray(inputs["Wv"], dtype=np.float32)
    Wout = np.asarray(inputs["Wout"], dtype=np.float32)    # [4096, 512]

    # Rank-D refactor: per head fold the QK^T and V-proj/out-proj pairs into
    # D x D matrices (U = 2D > D, so this more than halves the matmul work):
    #   scores = (z A + u) z^T      A = (g*Wq)(g*Wk)^T,  u = (b Wq)(g*Wk)^T
    #   head @ Wout = probs (z N) + (b Wv) Wout   N = (g*Wv) Wout
    # LN beta terms on the query side cancel in softmax; (b Wv) Wout is a
    # constant vector added host-side.
    H = 4
    A = np.empty((H, D, D), np.float32)
    Nm = np.empty((H, D, D), np.float32)
    ubias = np.empty((H, D), np.float32)
    cvec = np.zeros(D, np.float32)
    for h in range(H):
        Wkg = Wk[h] * gamma[:, None]
        A[h] = (Wq[h] * gamma[:, None]) @ Wkg.T
        ubias[h] = (beta @ Wq[h]) @ Wkg.T
        Nm[h] = (Wv[h] * gamma[:, None]) @ Wout[h * U:(h + 1) * U]
        cvec += (beta @ Wv[h]) @ Wout[h * U:(h + 1) * U]

    in_maps = []
    for c in range(8):
        b, hp = c // 2, c % 2
        ub = ubias[2 * hp:2 * hp + 2].reshape(2, ND, P).transpose(2, 0, 1).reshape(P, 2 * ND)
        in_maps.append({
            "x": np.ascontiguousarray(x[b]),
            "ub": np.ascontiguousarray(ub),
            "a": np.ascontiguousarray(A[2 * hp:2 * hp + 2].reshape(2 * D, D)).astype(np.float16),
            "n": np.ascontiguousarray(Nm[2 * hp:2 * hp + 2].reshape(2 * D, D)).astype(np.float16),
        })
    res = run_bass_kernel_spmd(_get_nc(), in_maps, list(range(8)), trace=trace)
    out = np.empty((4, S, D), np.float32)
    for b in range(4):
        out[b] = res.results[2 * b]["out"] + res.results[2 * b + 1]["out"] + cvec[None, :]
    return out, res


def kernel(**inputs):
    out, _ = _run(inputs, trace=False)
    return out
